# revision 10
# baseline (speedup 1.0000x reference)
# Trainium2 Bass kernel for nn_ConvAttentionHid (conv-attention with hidden gate).
#
# Reference computation (per batch b):
#   ctx  = context[b].T                      # [SRC, L]
#   L1   = relu(conv1d(ctx, w1, b1))         # [DIM1, L]
#   L2   = conv1d(L1, w2, b2) * input[b][:,None]
#   L2n  = L2 / ||L2||_rows                  # L2 normalize along L
#   attn = conv1d(L2n, w3, b3)               # [1, L]   (output 2)
#   a    = softmax(attn)
#   wc   = ctx @ a                           # [SRC]
#   h    = tanh(W_out @ [wc; input[b]])      # [TRG]    (output 1)
#
# Sharding: pure data parallelism, 4 batches per core on 8 cores.
# Self-contained: hardcodes all shapes; no sibling imports.

import os
import numpy as np
from contextlib import ExitStack

import concourse.bass as bass
import concourse.tile as tile
from concourse import bacc
from concourse import mybir
from concourse.bass_utils import run_bass_kernel_spmd
from concourse.masks import make_identity

F32 = mybir.dt.float32
BF16 = mybir.dt.bfloat16

N_CORES = 8
B = 32
PB = B // N_CORES          # batches per core (4)
L = 4096
C = 512                    # SRC channels
D = 512                    # DIM1
T = 512                    # TRG
KW = 3                     # conv kernel width
NT = 4                     # 128-tiles per 512 channels
HT = (T + C) // 128        # 8 h-dim tiles
LT = L // 128              # 32 L-tiles of 128
NS = L // 512              # 8 L-slices of 512
COLS = L + 2               # guarded length axis (zero col at 0 and L+1)

_compiled = None           # cache across kernel() calls


def build_program(phases=None):
    if phases is None:
        phases = int(os.environ.get("PHASES", "99"))
    nc = bacc.Bacc("TRN2", target_bir_lowering=False, debug=False,
                   num_devices=N_CORES)

    context_t = nc.declare_dram_parameter("context", [PB, L, C], F32, isOutput=False)
    input_t = nc.declare_dram_parameter("input", [PB, T], F32, isOutput=False)
    w1_t = nc.declare_dram_parameter("w1", [D, C, KW], F32, isOutput=False)
    b1_t = nc.declare_dram_parameter("b1", [D], F32, isOutput=False)
    w2_t = nc.declare_dram_parameter("w2", [T, D, KW], F32, isOutput=False)
    b2_t = nc.declare_dram_parameter("b2", [T], F32, isOutput=False)
    w3_t = nc.declare_dram_parameter("w3", [1, T, KW], F32, isOutput=False)
    b3_t = nc.declare_dram_parameter("b3", [1], F32, isOutput=False)
    wout_t = nc.declare_dram_parameter("W_out", [T, T + C], F32, isOutput=False)
    h_out = nc.declare_dram_parameter("h_out", [PB, T], F32, isOutput=True)
    attn_out = nc.declare_dram_parameter("attn_out", [PB, 1, L], F32, isOutput=True)

    with ExitStack() as ctx:
        tc = ctx.enter_context(tile.TileContext(nc))

        const = ctx.enter_context(tc.tile_pool(name="const", bufs=1))
        big = ctx.enter_context(tc.tile_pool(name="big", bufs=1))
        stage = ctx.enter_context(tc.tile_pool(name="stage", bufs=3))
        scratch = ctx.enter_context(tc.tile_pool(name="scratch", bufs=2))
        small = ctx.enter_context(tc.tile_pool(name="small", bufs=2))

        psum_mm = ctx.enter_context(tc.tile_pool(name="psum_mm", bufs=2, space="PSUM"))
        psum_tr = ctx.enter_context(tc.tile_pool(name="psum_tr", bufs=2, space="PSUM"))
        psum_p3 = ctx.enter_context(tc.tile_pool(name="psum_p3", bufs=2, space="PSUM"))
        psum_misc = ctx.enter_context(tc.tile_pool(name="psum_misc", bufs=2, space="PSUM"))

        # ---------------- constants ----------------
        ident_bf = const.tile([128, 128], BF16)
        make_identity(nc, ident_bf)

        b1col = const.tile([128, NT], F32)
        nc.sync.dma_start(out=b1col, in_=b1_t.ap().rearrange("(co p) -> p co", p=128))
        b2col = const.tile([128, NT], F32)
        nc.sync.dma_start(out=b2col, in_=b2_t.ap().rearrange("(co p) -> p co", p=128))
        w3_sb = const.tile([128, NT, KW], F32)
        nc.sync.dma_start(out=w3_sb, in_=w3_t.ap()[0].rearrange("(co p) k -> p co k", p=128))
        g_col = const.tile([128, PB, NT], F32)
        nc.sync.dma_start(out=g_col, in_=input_t.ap().rearrange("b (co p) -> p b co", p=128))
        b3b = const.tile([32, 1], F32)
        nc.sync.dma_start(out=b3b, in_=bass.AP(tensor=b3_t.ap().tensor, offset=0,
                                               ap=[[0, 32], [1, 1]]))
        ones32 = const.tile([32, 1], F32)
        nc.vector.memset(ones32, 1.0)

        # conv weights as lhsT tiles: wT[:, ci, k, co*128+j] = w[co*128+j, ci*128+:, k].T
        w1T = const.tile([128, NT, KW, 512], BF16)
        w2T = const.tile([128, NT, KW, 512], BF16)
        woutT = const.tile([128, HT, T], BF16)

        with tc.tile_pool(name="wstage", bufs=1) as wstage:
            for (w_dram, wT) in ((w1_t, w1T), (w2_t, w2T)):
                for co in range(NT):
                    wnat_f = wstage.tile([128, D * KW], F32, tag="wnat_f")
                    nc.sync.dma_start(out=wnat_f,
                                      in_=w_dram.ap()[co * 128:(co + 1) * 128, :, :])
                    wnat = wstage.tile([128, D, KW], BF16, tag="wnat")
                    nc.vector.tensor_copy(
                        out=wnat, in_=wnat_f.rearrange("p (ci k) -> p ci k", k=KW))
                    for ci in range(NT):
                        ps = psum_tr.tile([128, 512], BF16, tag="tr")
                        for k in range(KW):
                            nc.tensor.transpose(ps[:, k * 128:(k + 1) * 128],
                                                wnat[:, ci * 128:(ci + 1) * 128, k],
                                                ident_bf)
                        nc.scalar.copy(
                            out=wT[:, ci, :, co * 128:(co + 1) * 128],
                            in_=ps[:, 0:KW * 128].rearrange("p (k c) -> p k c", c=128))
            # W_out -> lhsT layout [hdim, trg]
            for m in range(NT):
                wof = wstage.tile([128, T + C], F32, tag="wof")
                nc.sync.dma_start(out=wof, in_=wout_t.ap()[m * 128:(m + 1) * 128, :])
                wob = wstage.tile([128, T + C], BF16, tag="wob")
                nc.vector.tensor_copy(out=wob, in_=wof)
                for t in range(HT):
                    ps = psum_tr.tile([128, 512], BF16, tag="tr")
                    nc.tensor.transpose(ps[:, m * 128:(m + 1) * 128],
                                        wob[:, t * 128:(t + 1) * 128], ident_bf)
                    nc.scalar.copy(out=woutT[:, t, m * 128:(m + 1) * 128],
                                   in_=ps[:, m * 128:(m + 1) * 128])

        # ---------------- persistent big buffers ----------------
        ctxT = big.tile([128, NT, COLS], BF16)    # [cin, L+2] guarded
        l1b = big.tile([128, NT, COLS], BF16)     # [dim1, L+2] guarded
        l2b = big.tile([128, NT, COLS], BF16)     # [trg, L+2] guarded
        ctx_nat = big.tile([128, LT, 512], BF16)  # [L-local, SRC] natural
        attn3 = big.tile([KW, COLS], BF16)        # 3 shifted attention rows
        hcol = const.tile([128, HT, PB], BF16)    # final matvec rhs

        for buf in (ctxT, l1b, l2b):
            nc.vector.memset(buf[:, :, 0:1], 0.0)
            nc.vector.memset(buf[:, :, COLS - 1:COLS], 0.0)
        nc.vector.memset(attn3[:, 0:1], 0.0)
        nc.vector.memset(attn3[:, COLS - 1:COLS], 0.0)

        # ---------------- per-batch pipeline ----------------
        for b in range(PB):
            if phases < 1:
                break
            # Phase 1: load context natural, cast bf16, transpose into ctxT
            for lt in range(LT):
                cf = stage.tile([128, 512], F32, tag="ctx_f")
                nc.sync.dma_start(out=cf,
                                  in_=context_t.ap()[b, lt * 128:(lt + 1) * 128, :])
                nc.vector.tensor_copy(out=ctx_nat[:, lt, :], in_=cf)
            for ci in range(NT):
                for lq in range(LT // 4):
                    ps = psum_tr.tile([128, 512], BF16, tag="tr")
                    for j in range(4):
                        lt = lq * 4 + j
                        nc.tensor.transpose(ps[:, j * 128:(j + 1) * 128],
                                            ctx_nat[:, lt, ci * 128:(ci + 1) * 128],
                                            ident_bf)
                    nc.scalar.copy(out=ctxT[:, ci, 1 + lq * 512: 1 + (lq + 1) * 512],
                                   in_=ps)

            # Phase 2: conv1 -> relu -> L1
            if phases < 2:
                continue
            for m in range(NT):
                for ns in range(NS):
                    ps = psum_mm.tile([128, 512], F32, tag="mm")
                    for i, (ci, k) in enumerate([(a, b_) for a in range(NT)
                                                 for b_ in range(KW)]):
                        nc.tensor.matmul(
                            ps,
                            w1T[:, ci, k, m * 128:(m + 1) * 128],
                            ctxT[:, ci, ns * 512 + k: ns * 512 + k + 512],
                            start=(i == 0), stop=(i == NT * KW - 1),
                        )
                    nc.scalar.activation(
                        out=l1b[:, m, 1 + ns * 512: 1 + (ns + 1) * 512],
                        in_=ps,
                        func=mybir.ActivationFunctionType.Relu,
                        bias=b1col[:, m:m + 1],
                    )

            # Phase 3: conv2 -> (x+b2)*g -> L2, with running sum of squares
            if phases < 3:
                continue
            accP = small.tile([128, NT, NS], F32, tag="accP")
            for m in range(NT):
                for ns in range(NS):
                    ps = psum_mm.tile([128, 512], F32, tag="mm")
                    for i, (ci, k) in enumerate([(a, b_) for a in range(NT)
                                                 for b_ in range(KW)]):
                        nc.tensor.matmul(
                            ps,
                            w2T[:, ci, k, m * 128:(m + 1) * 128],
                            l1b[:, ci, ns * 512 + k: ns * 512 + k + 512],
                            start=(i == 0), stop=(i == NT * KW - 1),
                        )
                    l2slice = l2b[:, m, 1 + ns * 512: 1 + (ns + 1) * 512]
                    nc.vector.tensor_scalar(
                        out=l2slice, in0=ps,
                        scalar1=b2col[:, m:m + 1], scalar2=g_col[:, b, m:m + 1],
                        op0=mybir.AluOpType.add, op1=mybir.AluOpType.mult,
                    )
                    sq = scratch.tile([128, 512], F32, tag="sq", bufs=1)
                    nc.scalar.activation(
                        out=sq, in_=l2slice,
                        func=mybir.ActivationFunctionType.Square,
                        accum_out=accP[:, m, ns:ns + 1],
                    )

            # Phase 4: w3n = w3 / ||L2||
            if phases < 4:
                continue
            acc = small.tile([128, NT], F32, tag="acc")
            nc.vector.reduce_sum(acc, accP, axis=mybir.AxisListType.X)
            normc = small.tile([128, NT], F32, tag="normc")
            nc.scalar.sqrt(normc, acc)
            rnorm = small.tile([128, NT], F32, tag="rnorm")
            nc.vector.reciprocal(rnorm, normc)
            w3n = small.tile([128, NT, KW], BF16, tag="w3n")
            for m in range(NT):
                nc.vector.tensor_scalar_mul(out=w3n[:, m, :], in0=w3_sb[:, m, :],
                                            scalar1=rnorm[:, m:m + 1])

            # Phase 5: 3-row attention matmuls over L2
            if phases < 5:
                continue
            for ns in range(NS):
                p3 = psum_p3.tile([KW, 512], F32, tag="p3")
                for m in range(NT):
                    nc.tensor.matmul(
                        p3,
                        w3n[:, m, :],
                        l2b[:, m, 1 + ns * 512: 1 + (ns + 1) * 512],
                        start=(m == 0), stop=(m == NT - 1),
                    )
                nc.vector.tensor_copy(out=attn3[:, 1 + ns * 512: 1 + (ns + 1) * 512],
                                      in_=p3)

            # Phase 6: combine shifted rows; attention output + softmax pieces
            if phases < 6:
                continue
            sh = []
            for j in range(KW):
                s = scratch.tile([32, 128], BF16, tag="sh", bufs=3)
                nc.sync.dma_start(out=s, in_=attn3[j:j + 1, j:j + L])
                sh.append(s)
            attn_nb = scratch.tile([32, 128], F32, tag="attn_nb")
            nc.vector.tensor_add(attn_nb, sh[0], sh[1])
            nc.vector.tensor_add(attn_nb, attn_nb, sh[2])
            attn_res = scratch.tile([32, 128], F32, tag="attn_res")
            nc.scalar.activation(out=attn_res, in_=attn_nb,
                                 func=mybir.ActivationFunctionType.Identity,
                                 bias=b3b)
            nc.sync.dma_start(
                out=attn_out.ap()[b, 0, :].rearrange("(p f) -> p f", p=32),
                in_=attn_res)
            # softmax numerator (constant b3 cancels; logits tiny -> no max-sub)
            e = scratch.tile([32, 128], F32, tag="e")
            nc.scalar.activation(out=e, in_=attn_nb,
                                 func=mybir.ActivationFunctionType.Exp)
            ered = small.tile([32, 1], F32, tag="ered")
            nc.vector.reduce_sum(ered, e, axis=mybir.AxisListType.X)
            psS = psum_misc.tile([1, 1], F32, tag="misc")
            nc.tensor.matmul(psS, ered, ones32, start=True, stop=True)
            rS = small.tile([1, 1], F32, tag="rS")
            nc.vector.reciprocal(rS, psS)

            e_bf = scratch.tile([32, 128], BF16, tag="e_bf")
            nc.vector.tensor_copy(out=e_bf, in_=e)
            psE = psum_misc.tile([128, 32], BF16, tag="misc")
            nc.tensor.transpose(psE, e_bf, ident_bf[0:32, 0:32])
            e_col = small.tile([128, 32], BF16, tag="e_col")
            nc.vector.tensor_copy(out=e_col, in_=psE)

            # Phase 7: weighted context wc[c] = sum_l ctx[l, c] * e[l], * 1/S
            if phases < 7:
                continue
            psW = psum_misc.tile([1, 512], F32, tag="misc")
            for lt in range(LT):
                nc.tensor.matmul(psW, e_col[:, lt:lt + 1], ctx_nat[:, lt, :],
                                 start=(lt == 0), stop=(lt == LT - 1))
            wc_bf = small.tile([1, 512], BF16, tag="wc_bf")
            nc.vector.tensor_scalar_mul(out=wc_bf, in0=psW, scalar1=rS)
            psH = psum_misc.tile([128, NT, 2], BF16, tag="misc")
            for t in range(NT):
                nc.tensor.transpose(psH[:, t, 0:1], wc_bf[0:1, t * 128:(t + 1) * 128],
                                    ident_bf[0:1, 0:1])
            nc.vector.tensor_copy(out=hcol[:, 0:NT, b], in_=psH[:, :, 0])
            nc.vector.tensor_copy(out=hcol[:, NT:HT, b], in_=g_col[:, b, :])

        # ---------------- final: h = tanh(W_out @ [wc; input]) ----------------
        for m in range(NT):
            if phases < 8:
                break
            psF = psum_misc.tile([128, PB], F32, tag="misc")
            for t in range(HT):
                nc.tensor.matmul(psF, woutT[:, t, m * 128:(m + 1) * 128],
                                 hcol[:, t, :],
                                 start=(t == 0), stop=(t == HT - 1))
            hsb = small.tile([128, PB], F32, tag="hsb")
            nc.scalar.activation(out=hsb, in_=psF,
                                 func=mybir.ActivationFunctionType.Tanh)
            nc.sync.dma_start(
                out=h_out.ap()[:, m * 128:(m + 1) * 128].rearrange("b p -> p b"),
                in_=hsb)

    nc.compile()
    return nc


def kernel(**inputs):
    global _compiled
    inp = {k: np.asarray(v, dtype=np.float32) for k, v in inputs.items()
           if k != "src_emb"}
    if _compiled is None:
        _compiled = build_program()
    nc = _compiled

    in_maps = []
    for c in range(N_CORES):
        sl = slice(c * PB, (c + 1) * PB)
        in_maps.append({
            "context": np.ascontiguousarray(inp["context"][sl]),
            "input": np.ascontiguousarray(inp["input"][sl]),
            "w1": inp["w1"], "b1": inp["b1"],
            "w2": inp["w2"], "b2": inp["b2"],
            "w3": inp["w3"], "b3": inp["b3"],
            "W_out": inp["W_out"],
        })
    res = run_bass_kernel_spmd(nc, in_maps, list(range(N_CORES)))
    h = np.concatenate([res.results[c]["h_out"] for c in range(N_CORES)], axis=0)
    attn = np.concatenate([res.results[c]["attn_out"] for c in range(N_CORES)], axis=0)
    return h, attn
